# revision 12
# baseline (speedup 1.0000x reference)
"""Trainium2 Bass kernel for nn_BeeGameModule (histogram_binning).

Reference computation (per batch b of 4096):
    agent_vote[b,a] = argmax_h votes[b,a,h]          (A=128 agents, H=64 hives)
    counts[b,h]     = histogram of agent_vote[b,:]
    max_freq[b]     = counts.max() / 128
    value[b]        = sum_a hive_values[b, agent_vote[b,a]]
                    = sum_h counts[b,h] * hive_values[b,h]
    discount[b]     = 100*(1 - sigmoid(30*(max_freq[b] - 0.7)))
    vote_cost       = -sum_b value[b]/discount[b]
    movement_cost   = sum_{b,e} ||movements[b,e,:]||_2
    out             = (vote_cost + movement_cost, max_freq)

Key identity: with no argmax ties (true in float32 for these inputs), the
one-hot eq[b,a,h] = (votes[b,a,h] == max_h votes[b,a,:]) gives counts exactly
via a sum over agents, and value via counts . hive_values. All counts are
small integers, exact in bf16/f32.

Layout: batch on partitions, two batches per partition row ("mega-tiles" of
256 batches), agents*hives contiguous in the free dim so the votes DMA is
fully contiguous. DVE does the segmented max + one-hot compare; GPSIMD takes
the first (largest) level of the agent tree-sum; DVE finishes the tree
in-place and the per-batch finalize; ACT does sigmoid/sqrt.

Sharding: pure data parallel over the batch axis across 8 cores; host sums
the 8 per-core scalar partials and concatenates max_freq shards.
"""

import numpy as np

B = 4096
A = 128          # agents
H = 64           # hives
E = 192          # entities
NCORES = 8
BC = B // NCORES         # 512 batches per core
P = 128                  # SBUF partitions
TWO = 2                  # batches per partition row
MT = BC // (P * TWO)     # 2 mega-tiles of 256 batches
D_DISC, K_DISC, T_DISC = 100.0, 30.0, 0.7

_CACHE = {}


def _build_bass(repeat: int = 1):
    import concourse.bass as bass
    import concourse.bacc as bacc
    import concourse.mybir as mybir
    from concourse.tile import TileContext
    from contextlib import ExitStack

    f32 = mybir.dt.float32
    bf16 = mybir.dt.bfloat16
    X = mybir.AxisListType.X
    Alu = mybir.AluOpType

    nc = bacc.Bacc()
    votes = nc.declare_dram_parameter("votes", [BC, A * H], f32, isOutput=False)
    mov = nc.declare_dram_parameter("movements", [BC, E * 2], f32, isOutput=False)
    hv = nc.declare_dram_parameter("hive_values", [BC, H], f32, isOutput=False)
    out_mf = nc.declare_dram_parameter("max_freq", [P, MT * TWO], f32, isOutput=True)
    out_part = nc.declare_dram_parameter("partial", [P], f32, isOutput=True)

    with TileContext(nc) as tc, ExitStack() as ctx:
        main = ctx.enter_context(tc.tile_pool(name="main", bufs=2))
        small = ctx.enter_context(tc.tile_pool(name="small", bufs=2))
        accp = ctx.enter_context(tc.tile_pool(name="accp", bufs=1))

        # acc: [0 : TWO*MT] vote contribs (col 2t+j), [TWO*MT :] movement sums
        acc = accp.tile([P, TWO * MT + MT], f32)
        mf_all = accp.tile([P, TWO * MT], f32)

        for rep in range(repeat):
            for t in range(MT):
                rows = slice(t * P * TWO, (t + 1) * P * TWO)

                # votes mega-tile: [128, 2 * 128 * 64], batches (2p+j) on row p
                v = main.tile([P, TWO * A * H], f32, tag="v", bufs=2,
                              name=f"v_{rep}_{t}")
                nc.sync.dma_start(
                    out=v,
                    in_=votes[rows].rearrange("(p two) f -> p (two f)", two=TWO))
                v4 = v.rearrange("p (a h) -> p a h", h=H)  # [P, 256, 64]

                # per-agent max over hives
                m = small.tile([P, TWO * A], f32, tag="m")
                nc.vector.reduce_max(m, v4, axis=X)

                # one-hot (exact): eq = (votes == m) as bf16
                mb = (m.rearrange("p (a o) -> p a o", o=1)
                      .broadcast_to([P, TWO * A, H]))
                eq = main.tile([P, TWO * A * H], bf16, tag="eq", bufs=1,
                               name=f"eq_{rep}_{t}")
                nc.vector.tensor_tensor(
                    eq.rearrange("p (a h) -> p a h", h=H), v4, mb, Alu.is_equal)

                # tree-sum over agents, per batch segment.
                # level 1 (largest) on GPSIMD, remaining levels on DVE in-place.
                eq3 = eq.rearrange("p (two f) -> p two f", two=TWO)
                half = A * H // 2
                t1 = main.tile([P, TWO * half], bf16, tag="t1", bufs=1,
                               name=f"t1_{rep}_{t}")
                t13 = t1.rearrange("p (two f) -> p two f", two=TWO)
                nc.gpsimd.tensor_tensor(t13, eq3[:, :, :half],
                                        eq3[:, :, half:2 * half], Alu.add)
                n = half // 2
                while n > H:
                    nc.vector.tensor_tensor(
                        t13[:, :, :n], t13[:, :, :n], t13[:, :, n:2 * n], Alu.add)
                    n //= 2
                counts = small.tile([P, TWO * H], f32, tag="counts")
                c3 = counts.rearrange("p (two h) -> p two h", two=TWO)
                nc.vector.tensor_tensor(c3, t13[:, :, :H], t13[:, :, H:2 * H],
                                        Alu.add)

                # max_freq = counts.max()/128  (per batch segment)
                mf_raw = small.tile([P, TWO], f32, tag="mf_raw")
                nc.vector.reduce_max(mf_raw, c3, axis=X)
                nc.scalar.mul(mf_all[:, TWO * t:TWO * (t + 1)], mf_raw, 1.0 / A)

                # value = sum_h counts*hv
                hvt = small.tile([P, TWO * H], f32, tag="hvt", bufs=2)
                nc.gpsimd.dma_start(
                    out=hvt,
                    in_=hv[rows].rearrange("(p two) h -> p (two h)", two=TWO))
                prod = small.tile([P, TWO * H], f32, tag="prod")
                nc.vector.tensor_tensor(
                    prod.rearrange("p (two h) -> p two h", two=TWO), c3,
                    hvt.rearrange("p (two h) -> p two h", two=TWO), Alu.mult)
                value = small.tile([P, TWO], f32, tag="value")
                nc.vector.reduce_sum(
                    value, prod.rearrange("p (two h) -> p two h", two=TWO),
                    axis=X)

                # discount = 100*(1 - sigmoid(30*(mf_raw/128 - 0.7)))
                sgarg = small.tile([P, TWO], f32, tag="sgarg")
                nc.vector.tensor_scalar(sgarg, mf_raw, K_DISC / A,
                                        -K_DISC * T_DISC, Alu.mult, Alu.add)
                sg = small.tile([P, TWO], f32, tag="sg")
                nc.scalar.activation(sg, sgarg,
                                     mybir.ActivationFunctionType.Sigmoid)
                denom = small.tile([P, TWO], f32, tag="denom")
                nc.vector.tensor_scalar(denom, sg, -D_DISC, D_DISC,
                                        Alu.mult, Alu.add)
                recip = small.tile([P, TWO], f32, tag="recip")
                nc.vector.reciprocal(recip, denom)
                nc.vector.tensor_tensor(acc[:, TWO * t:TWO * (t + 1)],
                                        value, recip, Alu.mult)

                # movement: sum of L2 norms (both batch segments summed together)
                mv = small.tile([P, TWO * E * 2], f32, tag="mv", bufs=2)
                nc.gpsimd.dma_start(
                    out=mv,
                    in_=mov[rows].rearrange("(p two) f -> p (two f)", two=TWO))
                sq = small.tile([P, TWO * E * 2], f32, tag="sq")
                nc.scalar.square(sq, mv)
                sq3 = sq.rearrange("p (e c) -> p e c", c=2)
                ps = small.tile([P, TWO * E], f32, tag="ps")
                nc.vector.tensor_tensor(ps, sq3[:, :, 0], sq3[:, :, 1], Alu.add)
                rt = small.tile([P, TWO * E], f32, tag="rt")
                nc.scalar.activation(
                    rt, ps, mybir.ActivationFunctionType.Sqrt,
                    accum_out=acc[:, TWO * MT + t:TWO * MT + t + 1])

        # per-partition total: sum(movement) - sum(value/discount)
        vsum = accp.tile([P, 1], f32)
        nc.vector.reduce_sum(
            vsum, acc[:, 0:TWO * MT].rearrange("p (o t) -> p o t", o=1), axis=X)
        msum = accp.tile([P, 1], f32)
        nc.vector.reduce_sum(
            msum, acc[:, TWO * MT:TWO * MT + MT]
            .rearrange("p (o t) -> p o t", o=1), axis=X)
        tot = accp.tile([P, 1], f32)
        nc.vector.tensor_tensor(tot, msum, vsum, Alu.subtract)
        nc.gpsimd.dma_start(out=out_part[:], in_=tot)
        # max_freq in raw [p, (t, j)] order; host reorders to batch order
        nc.sync.dma_start(out=out_mf[:], in_=mf_all)

    nc.finalize()
    return nc


def kernel(movements, utterances, votes, hive_values, locations):
    from concourse.bass_utils import run_bass_kernel_spmd

    if "nc" not in _CACHE:
        _CACHE["nc"] = _build_bass()
    nc = _CACHE["nc"]

    votes = np.ascontiguousarray(votes, dtype=np.float32)
    movements = np.ascontiguousarray(movements, dtype=np.float32)
    hive_values = np.ascontiguousarray(hive_values, dtype=np.float32)

    in_maps = []
    for c in range(NCORES):
        sl = slice(c * BC, (c + 1) * BC)
        in_maps.append({
            "votes": votes[sl].reshape(BC, A * H),
            "movements": movements[sl].reshape(BC, E * 2),
            "hive_values": hive_values[sl].reshape(BC, H),
        })

    res = run_bass_kernel_spmd(nc, in_maps, core_ids=list(range(NCORES)))
    _CACHE["last_result"] = res

    # raw[p, t, j] holds batch t*256 + 2p + j of the core's shard
    max_freq = np.concatenate([
        r["max_freq"].reshape(P, MT, TWO).transpose(1, 0, 2).reshape(BC)
        for r in res.results])
    total = np.float32(np.sum(np.float64(
        np.concatenate([r["partial"] for r in res.results]))))
    return (total, max_freq)


# revision 13
# speedup vs baseline: 1.0375x; 1.0375x over previous
"""Trainium2 Bass kernel for nn_BeeGameModule (histogram_binning).

Reference computation (per batch b of 4096):
    agent_vote[b,a] = argmax_h votes[b,a,h]          (A=128 agents, H=64 hives)
    counts[b,h]     = histogram of agent_vote[b,:]
    max_freq[b]     = counts.max() / 128
    value[b]        = sum_a hive_values[b, agent_vote[b,a]]
                    = sum_h counts[b,h] * hive_values[b,h]
    discount[b]     = 100*(1 - sigmoid(30*(max_freq[b] - 0.7)))
    vote_cost       = -sum_b value[b]/discount[b]
    movement_cost   = sum_{b,e} ||movements[b,e,:]||_2
    out             = (vote_cost + movement_cost, max_freq)

Key identity: with no argmax ties (true in float32 for these inputs), the
one-hot eq[b,a,h] = (votes[b,a,h] == max_h votes[b,a,:]) gives counts exactly
via a sum over agents, and value via counts . hive_values. All counts are
small integers, exact in bf16/f32.

Layout: batch on partitions, two batches per partition row ("mega-tiles" of
256 batches), agents*hives contiguous in the free dim so the votes DMA is
fully contiguous. DVE does the segmented max + one-hot compare; GPSIMD takes
the first (largest) level of the agent tree-sum; DVE finishes the tree
in-place and the per-batch finalize; ACT does sigmoid/sqrt.

Sharding: pure data parallel over the batch axis across 8 cores; host sums
the 8 per-core scalar partials and concatenates max_freq shards.
"""

import numpy as np

B = 4096
A = 128          # agents
H = 64           # hives
E = 192          # entities
NCORES = 8
BC = B // NCORES         # 512 batches per core
P = 128                  # SBUF partitions
TWO = 2                  # batches per partition row
MT = BC // (P * TWO)     # 2 mega-tiles of 256 batches
D_DISC, K_DISC, T_DISC = 100.0, 30.0, 0.7

_CACHE = {}


def _build_bass(repeat: int = 1):
    import concourse.bass as bass
    import concourse.bacc as bacc
    import concourse.mybir as mybir
    from concourse.tile import TileContext
    from contextlib import ExitStack

    f32 = mybir.dt.float32
    bf16 = mybir.dt.bfloat16
    X = mybir.AxisListType.X
    Alu = mybir.AluOpType

    nc = bacc.Bacc()
    votes = nc.declare_dram_parameter("votes", [BC, A * H], f32, isOutput=False)
    mov = nc.declare_dram_parameter("movements", [BC, E * 2], f32, isOutput=False)
    hv = nc.declare_dram_parameter("hive_values", [BC, H], f32, isOutput=False)
    out_mf = nc.declare_dram_parameter("max_freq", [P, MT * TWO], f32, isOutput=True)
    out_part = nc.declare_dram_parameter("partial", [P], f32, isOutput=True)

    with TileContext(nc) as tc, ExitStack() as ctx:
        main = ctx.enter_context(tc.tile_pool(name="main", bufs=2))
        small = ctx.enter_context(tc.tile_pool(name="small", bufs=2))
        accp = ctx.enter_context(tc.tile_pool(name="accp", bufs=1))

        # acc: [0 : TWO*MT] vote contribs (col 2t+j), [TWO*MT :] movement sums
        acc = accp.tile([P, TWO * MT + MT], f32)
        mf_all = accp.tile([P, TWO * MT], f32)
        bias_sg = accp.tile([P, 1], f32)
        nc.vector.memset(bias_sg, -K_DISC * T_DISC)
        bias_dn = accp.tile([P, 1], f32)
        nc.vector.memset(bias_dn, D_DISC)

        for rep in range(repeat):
            for t in range(MT):
                rows = slice(t * P * TWO, (t + 1) * P * TWO)

                # votes mega-tile: [128, 2 * 128 * 64], batches (2p+j) on row p
                v = main.tile([P, TWO * A * H], f32, tag="v", bufs=2,
                              name=f"v_{rep}_{t}")
                nc.sync.dma_start(
                    out=v,
                    in_=votes[rows].rearrange("(p two) f -> p (two f)", two=TWO))
                v4 = v.rearrange("p (a h) -> p a h", h=H)  # [P, 256, 64]

                # per-agent max over hives
                m = small.tile([P, TWO * A], f32, tag="m")
                nc.vector.reduce_max(m, v4, axis=X)

                # one-hot (exact): eq = (votes == m) as bf16
                mb = (m.rearrange("p (a o) -> p a o", o=1)
                      .broadcast_to([P, TWO * A, H]))
                eq = main.tile([P, TWO * A * H], bf16, tag="eq", bufs=1,
                               name=f"eq_{rep}_{t}")
                nc.vector.tensor_tensor(
                    eq.rearrange("p (a h) -> p a h", h=H), v4, mb, Alu.is_equal)

                # tree-sum over agents, per batch segment.
                # level 1 (largest) on GPSIMD, remaining levels on DVE in-place.
                eq3 = eq.rearrange("p (two f) -> p two f", two=TWO)
                half = A * H // 2
                t1 = main.tile([P, TWO * half], bf16, tag="t1", bufs=1,
                               name=f"t1_{rep}_{t}")
                t13 = t1.rearrange("p (two f) -> p two f", two=TWO)
                nc.gpsimd.tensor_tensor(t13, eq3[:, :, :half],
                                        eq3[:, :, half:2 * half], Alu.add)
                n = half // 2
                while n > H:
                    nc.vector.tensor_tensor(
                        t13[:, :, :n], t13[:, :, :n], t13[:, :, n:2 * n], Alu.add)
                    n //= 2
                counts = small.tile([P, TWO * H], f32, tag="counts")
                c3 = counts.rearrange("p (two h) -> p two h", two=TWO)
                nc.vector.tensor_tensor(c3, t13[:, :, :H], t13[:, :, H:2 * H],
                                        Alu.add)

                # max_freq = counts.max()/128  (per batch segment)
                mf_raw = small.tile([P, TWO], f32, tag="mf_raw")
                nc.vector.reduce_max(mf_raw, c3, axis=X)
                nc.scalar.mul(mf_all[:, TWO * t:TWO * (t + 1)], mf_raw, 1.0 / A)

                # value = sum_h counts*hv
                hvt = small.tile([P, TWO * H], f32, tag="hvt", bufs=2)
                nc.scalar.dma_start(
                    out=hvt,
                    in_=hv[rows].rearrange("(p two) h -> p (two h)", two=TWO))
                prod = small.tile([P, TWO * H], f32, tag="prod")
                nc.gpsimd.tensor_tensor(
                    prod.rearrange("p (two h) -> p two h", two=TWO), c3,
                    hvt.rearrange("p (two h) -> p two h", two=TWO), Alu.mult)
                value = small.tile([P, TWO], f32, tag="value")
                nc.vector.reduce_sum(
                    value, prod.rearrange("p (two h) -> p two h", two=TWO),
                    axis=X)

                # discount = 100*(1 - sigmoid(30*(mf_raw/128 - 0.7)))
                sg = small.tile([P, TWO], f32, tag="sg")
                nc.scalar.activation(sg, mf_raw,
                                     mybir.ActivationFunctionType.Sigmoid,
                                     bias=bias_sg[:, 0:1], scale=K_DISC / A)
                denom = small.tile([P, TWO], f32, tag="denom")
                nc.scalar.activation(denom, sg,
                                     mybir.ActivationFunctionType.Identity,
                                     bias=bias_dn[:, 0:1], scale=-D_DISC)
                recip = small.tile([P, TWO], f32, tag="recip")
                nc.vector.reciprocal(recip, denom)
                nc.vector.tensor_tensor(acc[:, TWO * t:TWO * (t + 1)],
                                        value, recip, Alu.mult)

                # movement: sum of L2 norms (both batch segments summed together)
                mv = small.tile([P, TWO * E * 2], f32, tag="mv", bufs=2)
                nc.scalar.dma_start(
                    out=mv,
                    in_=mov[rows].rearrange("(p two) f -> p (two f)", two=TWO))
                sq = small.tile([P, TWO * E * 2], f32, tag="sq")
                nc.scalar.square(sq, mv)
                sq3 = sq.rearrange("p (e c) -> p e c", c=2)
                ps = small.tile([P, TWO * E], f32, tag="ps")
                nc.gpsimd.tensor_tensor(ps, sq3[:, :, 0], sq3[:, :, 1], Alu.add)
                rt = small.tile([P, TWO * E], f32, tag="rt")
                nc.scalar.activation(
                    rt, ps, mybir.ActivationFunctionType.Sqrt,
                    accum_out=acc[:, TWO * MT + t:TWO * MT + t + 1])

        # per-partition total: sum(movement) - sum(value/discount)
        vsum = accp.tile([P, 1], f32)
        nc.vector.reduce_sum(
            vsum, acc[:, 0:TWO * MT].rearrange("p (o t) -> p o t", o=1), axis=X)
        msum = accp.tile([P, 1], f32)
        nc.vector.reduce_sum(
            msum, acc[:, TWO * MT:TWO * MT + MT]
            .rearrange("p (o t) -> p o t", o=1), axis=X)
        tot = accp.tile([P, 1], f32)
        nc.vector.tensor_tensor(tot, msum, vsum, Alu.subtract)
        nc.gpsimd.dma_start(out=out_part[:], in_=tot)
        # max_freq in raw [p, (t, j)] order; host reorders to batch order
        nc.sync.dma_start(out=out_mf[:], in_=mf_all)

    nc.finalize()
    return nc


def kernel(movements, utterances, votes, hive_values, locations):
    from concourse.bass_utils import run_bass_kernel_spmd

    if "nc" not in _CACHE:
        _CACHE["nc"] = _build_bass()
    nc = _CACHE["nc"]

    votes = np.ascontiguousarray(votes, dtype=np.float32)
    movements = np.ascontiguousarray(movements, dtype=np.float32)
    hive_values = np.ascontiguousarray(hive_values, dtype=np.float32)

    in_maps = []
    for c in range(NCORES):
        sl = slice(c * BC, (c + 1) * BC)
        in_maps.append({
            "votes": votes[sl].reshape(BC, A * H),
            "movements": movements[sl].reshape(BC, E * 2),
            "hive_values": hive_values[sl].reshape(BC, H),
        })

    res = run_bass_kernel_spmd(nc, in_maps, core_ids=list(range(NCORES)))
    _CACHE["last_result"] = res

    # raw[p, t, j] holds batch t*256 + 2p + j of the core's shard
    max_freq = np.concatenate([
        r["max_freq"].reshape(P, MT, TWO).transpose(1, 0, 2).reshape(BC)
        for r in res.results])
    total = np.float32(np.sum(np.float64(
        np.concatenate([r["partial"] for r in res.results]))))
    return (total, max_freq)


# revision 16
# speedup vs baseline: 1.3619x; 1.3127x over previous
"""Trainium2 Bass kernel for nn_BeeGameModule (histogram_binning).

Reference computation (per batch b of 4096):
    agent_vote[b,a] = argmax_h votes[b,a,h]          (A=128 agents, H=64 hives)
    counts[b,h]     = histogram of agent_vote[b,:]
    max_freq[b]     = counts.max() / 128
    value[b]        = sum_a hive_values[b, agent_vote[b,a]]
                    = sum_h counts[b,h] * hive_values[b,h]
    discount[b]     = 100*(1 - sigmoid(30*(max_freq[b] - 0.7)))
    vote_cost       = -sum_b value[b]/discount[b]
    movement_cost   = sum_{b,e} ||movements[b,e,:]||_2
    out             = (vote_cost + movement_cost, max_freq)

Key identity: with no argmax ties (true in float32 for these inputs), the
one-hot eq[b,a,h] = (votes[b,a,h] == max_h votes[b,a,:]) gives counts exactly
via a sum over agents, and value via counts . hive_values. All counts are
small integers, exact in bf16/f32.

Layout: batch on partitions, two batches per partition row ("mega-tiles" of
256 batches), agents*hives contiguous in the free dim so the votes DMA is
fully contiguous. DVE does the segmented max + one-hot compare; GPSIMD takes
the first (largest) level of the agent tree-sum; DVE finishes the tree
in-place and the per-batch finalize; ACT does sigmoid/sqrt.

Sharding: pure data parallel over the batch axis across 8 cores; host sums
the 8 per-core scalar partials and concatenates max_freq shards.
"""

import numpy as np

B = 4096
A = 128          # agents
H = 64           # hives
E = 192          # entities
NCORES = 8
BC = B // NCORES         # 512 batches per core
P = 128                  # SBUF partitions
TWO = 2                  # batches per partition row
MT = BC // (P * TWO)     # 2 mega-tiles of 256 batches
D_DISC, K_DISC, T_DISC = 100.0, 30.0, 0.7

_CACHE = {}


def _build_bass(repeat: int = 1):
    import concourse.bass as bass
    import concourse.bacc as bacc
    import concourse.mybir as mybir
    from concourse.tile import TileContext
    from contextlib import ExitStack

    f32 = mybir.dt.float32
    bf16 = mybir.dt.bfloat16
    X = mybir.AxisListType.X
    Alu = mybir.AluOpType

    nc = bacc.Bacc()
    votes = nc.declare_dram_parameter("votes", [BC, A * H], f32, isOutput=False)
    mov = nc.declare_dram_parameter("movements", [BC, E * 2], f32, isOutput=False)
    hv = nc.declare_dram_parameter("hive_values", [BC, H], f32, isOutput=False)
    out_mf = nc.declare_dram_parameter("max_freq", [P, MT * TWO], f32, isOutput=True)
    out_part = nc.declare_dram_parameter("partial", [P], f32, isOutput=True)

    with TileContext(nc) as tc, ExitStack() as ctx:
        main = ctx.enter_context(tc.tile_pool(name="main", bufs=2))
        small = ctx.enter_context(tc.tile_pool(name="small", bufs=2))
        accp = ctx.enter_context(tc.tile_pool(name="accp", bufs=1))

        # acc: [0 : TWO*MT] vote contribs (col 2t+j), [TWO*MT :] movement sums
        acc = accp.tile([P, TWO * MT + MT], f32)
        mf_all = accp.tile([P, TWO * MT], f32)
        bias_sg = accp.tile([P, 1], f32)
        nc.vector.memset(bias_sg, -K_DISC * T_DISC)
        bias_dn = accp.tile([P, 1], f32)
        nc.vector.memset(bias_dn, D_DISC)
        S = MT * TWO  # batch segments per partition across all tiles
        counts_all = accp.tile([P, S * H], f32)
        hvt_all = accp.tile([P, S * H], f32)
        for t in range(MT):
            rows = slice(t * P * TWO, (t + 1) * P * TWO)
            nc.scalar.dma_start(
                out=hvt_all[:, t * TWO * H:(t + 1) * TWO * H],
                in_=hv[rows].rearrange("(p two) h -> p (two h)", two=TWO))

        for rep in range(repeat):
            for t in range(MT):
                rows = slice(t * P * TWO, (t + 1) * P * TWO)

                # votes mega-tile: [128, 2 * 128 * 64], batches (2p+j) on row p
                v = main.tile([P, TWO * A * H], f32, tag="v", bufs=2,
                              name=f"v_{rep}_{t}")
                nc.sync.dma_start(
                    out=v,
                    in_=votes[rows].rearrange("(p two) f -> p (two f)", two=TWO))
                v4 = v.rearrange("p (a h) -> p a h", h=H)  # [P, 256, 64]

                # per-agent max over hives
                m = small.tile([P, TWO * A], f32, tag="m")
                nc.vector.reduce_max(m, v4, axis=X)

                # one-hot (exact): eq = (votes == m) as bf16
                mb = (m.rearrange("p (a o) -> p a o", o=1)
                      .broadcast_to([P, TWO * A, H]))
                eq = main.tile([P, TWO * A * H], bf16, tag="eq", bufs=2,
                               name=f"eq_{rep}_{t}")
                nc.vector.tensor_tensor(
                    eq.rearrange("p (a h) -> p a h", h=H), v4, mb, Alu.is_equal)

                # tree-sum over agents, per batch segment: all levels in-place
                # inside eq (level 1, the largest, on GPSIMD; rest on DVE).
                eq3 = eq.rearrange("p (two f) -> p two f", two=TWO)
                n = A * H // 2
                nc.gpsimd.tensor_tensor(eq3[:, :, :n], eq3[:, :, :n],
                                        eq3[:, :, n:2 * n], Alu.add)
                n //= 2
                while n > H:
                    nc.vector.tensor_tensor(
                        eq3[:, :, :n], eq3[:, :, :n], eq3[:, :, n:2 * n], Alu.add)
                    n //= 2
                c3 = (counts_all[:, t * TWO * H:(t + 1) * TWO * H]
                      .rearrange("p (two h) -> p two h", two=TWO))
                nc.vector.tensor_tensor(c3, eq3[:, :, :H], eq3[:, :, H:2 * H],
                                        Alu.add)

                # movement: sum of L2 norms (both batch segments summed together)
                mv = small.tile([P, TWO * E * 2], f32, tag="mv", bufs=2)
                nc.scalar.dma_start(
                    out=mv,
                    in_=mov[rows].rearrange("(p two) f -> p (two f)", two=TWO))
                nc.scalar.square(mv, mv)
                sq3 = mv.rearrange("p (e c) -> p e c", c=2)
                ps = small.tile([P, TWO * E], f32, tag="ps")
                nc.gpsimd.tensor_tensor(ps, sq3[:, :, 0], sq3[:, :, 1], Alu.add)
                nc.scalar.activation(
                    ps, ps, mybir.ActivationFunctionType.Sqrt,
                    accum_out=acc[:, TWO * MT + t:TWO * MT + t + 1])

        # batched finalize over all segments: [P, S, H]
        c4 = counts_all.rearrange("p (s h) -> p s h", h=H)
        mf_raw = accp.tile([P, S], f32)
        nc.vector.reduce_max(mf_raw, c4, axis=X)
        nc.scalar.mul(mf_all, mf_raw, 1.0 / A)
        prod = accp.tile([P, S * H], f32)
        nc.gpsimd.tensor_tensor(
            prod.rearrange("p (s h) -> p s h", h=H), c4,
            hvt_all.rearrange("p (s h) -> p s h", h=H), Alu.mult)
        value = accp.tile([P, S], f32)
        nc.vector.reduce_sum(value, prod.rearrange("p (s h) -> p s h", h=H),
                             axis=X)
        sg = accp.tile([P, S], f32)
        nc.scalar.activation(sg, mf_raw, mybir.ActivationFunctionType.Sigmoid,
                             bias=bias_sg[:, 0:1], scale=K_DISC / A)
        denom = accp.tile([P, S], f32)
        nc.scalar.activation(denom, sg,
                             mybir.ActivationFunctionType.Identity,
                             bias=bias_dn[:, 0:1], scale=-D_DISC)
        recip = accp.tile([P, S], f32)
        nc.vector.reciprocal(recip, denom)
        nc.vector.tensor_tensor(acc[:, 0:S], value, recip, Alu.mult)

        # per-partition total: sum(movement) - sum(value/discount)
        vsum = accp.tile([P, 1], f32)
        nc.vector.reduce_sum(
            vsum, acc[:, 0:TWO * MT].rearrange("p (o t) -> p o t", o=1), axis=X)
        msum = accp.tile([P, 1], f32)
        nc.vector.reduce_sum(
            msum, acc[:, TWO * MT:TWO * MT + MT]
            .rearrange("p (o t) -> p o t", o=1), axis=X)
        tot = accp.tile([P, 1], f32)
        nc.vector.tensor_tensor(tot, msum, vsum, Alu.subtract)
        nc.gpsimd.dma_start(out=out_part[:], in_=tot)
        # max_freq in raw [p, (t, j)] order; host reorders to batch order
        nc.sync.dma_start(out=out_mf[:], in_=mf_all)

    nc.finalize()
    return nc


def kernel(movements, utterances, votes, hive_values, locations):
    from concourse.bass_utils import run_bass_kernel_spmd

    if "nc" not in _CACHE:
        _CACHE["nc"] = _build_bass()
    nc = _CACHE["nc"]

    votes = np.ascontiguousarray(votes, dtype=np.float32)
    movements = np.ascontiguousarray(movements, dtype=np.float32)
    hive_values = np.ascontiguousarray(hive_values, dtype=np.float32)

    in_maps = []
    for c in range(NCORES):
        sl = slice(c * BC, (c + 1) * BC)
        in_maps.append({
            "votes": votes[sl].reshape(BC, A * H),
            "movements": movements[sl].reshape(BC, E * 2),
            "hive_values": hive_values[sl].reshape(BC, H),
        })

    res = run_bass_kernel_spmd(nc, in_maps, core_ids=list(range(NCORES)))
    _CACHE["last_result"] = res

    # raw[p, t, j] holds batch t*256 + 2p + j of the core's shard
    max_freq = np.concatenate([
        r["max_freq"].reshape(P, MT, TWO).transpose(1, 0, 2).reshape(BC)
        for r in res.results])
    total = np.float32(np.sum(np.float64(
        np.concatenate([r["partial"] for r in res.results]))))
    return (total, max_freq)


# revision 19
# speedup vs baseline: 1.5310x; 1.1241x over previous
"""Trainium2 Bass kernel for nn_BeeGameModule (histogram_binning).

Reference computation (per batch b of 4096):
    agent_vote[b,a] = argmax_h votes[b,a,h]          (A=128 agents, H=64 hives)
    counts[b,h]     = histogram of agent_vote[b,:]
    max_freq[b]     = counts.max() / 128
    value[b]        = sum_a hive_values[b, agent_vote[b,a]]
                    = sum_h counts[b,h] * hive_values[b,h]
    discount[b]     = 100*(1 - sigmoid(30*(max_freq[b] - 0.7)))
    vote_cost       = -sum_b value[b]/discount[b]
    movement_cost   = sum_{b,e} ||movements[b,e,:]||_2
    out             = (vote_cost + movement_cost, max_freq)

Key identity: with no argmax ties (true in float32 for these inputs), the
one-hot eq[b,a,h] = (votes[b,a,h] == max_h votes[b,a,:]) gives counts exactly
via a sum over agents, and value via counts . hive_values. All counts are
small integers, exact in bf16/f32.

Layout: batch on partitions, two batches per partition row ("mega-tiles" of
256 batches), agents*hives contiguous in the free dim so the votes DMA is
fully contiguous. DVE does the segmented max + one-hot compare; GPSIMD takes
the first (largest) level of the agent tree-sum; DVE finishes the tree
in-place and the per-batch finalize; ACT does sigmoid/sqrt.

Sharding: pure data parallel over the batch axis across 8 cores; host sums
the 8 per-core scalar partials and concatenates max_freq shards.
"""

import numpy as np

B = 4096
A = 128          # agents
H = 64           # hives
E = 192          # entities
NCORES = 8
BC = B // NCORES         # 512 batches per core
P = 128                  # SBUF partitions
TWO = 2                  # batches per partition row
MT = BC // (P * TWO)     # 2 mega-tiles of 256 batches
D_DISC, K_DISC, T_DISC = 100.0, 30.0, 0.7

_CACHE = {}


def _build_bass(repeat: int = 1):
    import concourse.bass as bass
    import concourse.bacc as bacc
    import concourse.mybir as mybir
    from concourse.tile import TileContext
    from contextlib import ExitStack

    f32 = mybir.dt.float32
    bf16 = mybir.dt.bfloat16
    X = mybir.AxisListType.X
    Alu = mybir.AluOpType

    nc = bacc.Bacc()
    votes = nc.declare_dram_parameter("votes", [BC, A * H], f32, isOutput=False)
    mov = nc.declare_dram_parameter("movements", [BC, E * 2], f32, isOutput=False)
    hv = nc.declare_dram_parameter("hive_values", [BC, H], f32, isOutput=False)
    out_mf = nc.declare_dram_parameter("max_freq", [P, MT * TWO], f32, isOutput=True)
    out_part = nc.declare_dram_parameter("partial", [P], f32, isOutput=True)

    with TileContext(nc) as tc, ExitStack() as ctx:
        main = ctx.enter_context(tc.tile_pool(name="main", bufs=2))
        small = ctx.enter_context(tc.tile_pool(name="small", bufs=2))
        accp = ctx.enter_context(tc.tile_pool(name="accp", bufs=1))

        # acc: [0 : TWO*MT] vote contribs (col 2t+j), [TWO*MT :] movement sums
        acc = accp.tile([P, TWO * MT + MT], f32)
        mf_all = accp.tile([P, TWO * MT], f32)
        bias_sg = accp.tile([P, 1], f32)
        nc.vector.memset(bias_sg, -K_DISC * T_DISC)
        bias_dn = accp.tile([P, 1], f32)
        nc.vector.memset(bias_dn, D_DISC)
        S = MT * TWO  # batch segments per partition across all tiles
        counts_all = accp.tile([P, S * H], f32)
        hvt_all = accp.tile([P, S * H], f32)
        for t in range(MT):
            rows = slice(t * P * TWO, (t + 1) * P * TWO)
            nc.scalar.dma_start(
                out=hvt_all[:, t * TWO * H:(t + 1) * TWO * H],
                in_=hv[rows].rearrange("(p two) h -> p (two h)", two=TWO))

        for rep in range(repeat):
            for t in range(MT):
                rows = slice(t * P * TWO, (t + 1) * P * TWO)

                # votes mega-tile: [128, 2 * 128 * 64], batches (2p+j) on row p
                v = main.tile([P, TWO * A * H], f32, tag="v", bufs=2,
                              name=f"v_{rep}_{t}")
                nc.sync.dma_start(
                    out=v,
                    in_=votes[rows].rearrange("(p two) f -> p (two f)", two=TWO))
                v4 = v.rearrange("p (a h) -> p a h", h=H)  # [P, 256, 64]

                # per-agent max over hives
                m = small.tile([P, TWO * A], f32, tag="m")
                nc.vector.reduce_max(m, v4, axis=X)

                # one-hot (exact): eq = (votes == m) as bf16
                mb = (m.rearrange("p (a o) -> p a o", o=1)
                      .broadcast_to([P, TWO * A, H]))
                eq = main.tile([P, TWO * A * H], bf16, tag="eq", bufs=2,
                               name=f"eq_{rep}_{t}")
                nc.vector.tensor_tensor(
                    eq.rearrange("p (a h) -> p a h", h=H), v4, mb, Alu.is_equal)

                # tree-sum over agents, per batch segment: all levels in-place
                # inside eq (level 1, the largest, on GPSIMD; rest on DVE).
                eq3 = eq.rearrange("p (two f) -> p two f", two=TWO)
                n = A * H // 2
                nc.gpsimd.tensor_tensor(eq3[:, :, :n], eq3[:, :, :n],
                                        eq3[:, :, n:2 * n], Alu.add)
                n //= 2
                while n > H:
                    nc.vector.tensor_tensor(
                        eq3[:, :, :n], eq3[:, :, :n], eq3[:, :, n:2 * n], Alu.add)
                    n //= 2
                c3 = (counts_all[:, t * TWO * H:(t + 1) * TWO * H]
                      .rearrange("p (two h) -> p two h", two=TWO))
                nc.vector.tensor_tensor(c3, eq3[:, :, :H], eq3[:, :, H:2 * H],
                                        Alu.add)

                # movement: sum of L2 norms (both batch segments summed together)
                mv = small.tile([P, TWO * E * 2], f32, tag="mv", bufs=2)
                nc.scalar.dma_start(
                    out=mv,
                    in_=mov[rows].rearrange("(p two) f -> p (two f)", two=TWO))
                nc.scalar.square(mv, mv)
                sq3 = mv.rearrange("p (e c) -> p e c", c=2)
                ps = small.tile([P, TWO * E], f32, tag="ps")
                nc.gpsimd.tensor_tensor(ps, sq3[:, :, 0], sq3[:, :, 1], Alu.add)
                nc.scalar.activation(
                    ps, ps, mybir.ActivationFunctionType.Sqrt,
                    accum_out=acc[:, TWO * MT + t:TWO * MT + t + 1])

        # batched finalize over all segments: [P, S, H]
        c4 = counts_all.rearrange("p (s h) -> p s h", h=H)
        mf_raw = accp.tile([P, S], f32)
        nc.vector.reduce_max(mf_raw, c4, axis=X)
        nc.scalar.mul(mf_all, mf_raw, 1.0 / A)
        prod = accp.tile([P, S * H], f32)
        nc.gpsimd.tensor_tensor(
            prod.rearrange("p (s h) -> p s h", h=H), c4,
            hvt_all.rearrange("p (s h) -> p s h", h=H), Alu.mult)
        value = accp.tile([P, S], f32)
        nc.vector.reduce_sum(value, prod.rearrange("p (s h) -> p s h", h=H),
                             axis=X)
        sg = accp.tile([P, S], f32)
        nc.scalar.activation(sg, mf_raw, mybir.ActivationFunctionType.Sigmoid,
                             bias=bias_sg[:, 0:1], scale=K_DISC / A)
        denom = accp.tile([P, S], f32)
        nc.scalar.activation(denom, sg,
                             mybir.ActivationFunctionType.Identity,
                             bias=bias_dn[:, 0:1], scale=-D_DISC)
        recip = accp.tile([P, S], f32)
        nc.vector.reciprocal(recip, denom)
        # write NEGATED vote contribs so one reduce over all of acc gives
        # sum(movement) - sum(value/discount) directly
        nrec = accp.tile([P, S], f32)
        nc.vector.tensor_scalar(nrec, recip, -1.0, 0.0, Alu.mult, Alu.add)
        nc.vector.tensor_tensor(acc[:, 0:S], value, nrec, Alu.mult)
        tot = accp.tile([P, 1], f32)
        nc.vector.reduce_sum(
            tot, acc.rearrange("p (o t) -> p o t", o=1), axis=X)
        nc.gpsimd.dma_start(out=out_part[:], in_=tot)
        # max_freq in raw [p, (t, j)] order; host reorders to batch order
        nc.sync.dma_start(out=out_mf[:], in_=mf_all)

    nc.finalize()
    return nc


def kernel(movements, utterances, votes, hive_values, locations):
    from concourse.bass_utils import run_bass_kernel_spmd

    if "nc" not in _CACHE:
        _CACHE["nc"] = _build_bass()
    nc = _CACHE["nc"]

    votes = np.ascontiguousarray(votes, dtype=np.float32)
    movements = np.ascontiguousarray(movements, dtype=np.float32)
    hive_values = np.ascontiguousarray(hive_values, dtype=np.float32)

    in_maps = []
    for c in range(NCORES):
        sl = slice(c * BC, (c + 1) * BC)
        in_maps.append({
            "votes": votes[sl].reshape(BC, A * H),
            "movements": movements[sl].reshape(BC, E * 2),
            "hive_values": hive_values[sl].reshape(BC, H),
        })

    res = run_bass_kernel_spmd(nc, in_maps, core_ids=list(range(NCORES)))
    _CACHE["last_result"] = res

    # raw[p, t, j] holds batch t*256 + 2p + j of the core's shard
    max_freq = np.concatenate([
        r["max_freq"].reshape(P, MT, TWO).transpose(1, 0, 2).reshape(BC)
        for r in res.results])
    total = np.float32(np.sum(np.float64(
        np.concatenate([r["partial"] for r in res.results]))))
    return (total, max_freq)
